# revision 8
# baseline (speedup 1.0000x reference)
"""Fused transformer layer (LN->attn->LN->MLP, residuals) on 8 NeuronCores.

Sharding: pure sequence/data parallel - core c handles batch c//4, query
tokens (c%4)*512..+512. The reference mask allows key j iff j <= q%1024, so
only keys 0..1023 of each batch are ever attended; each core computes k/v
for those 1024 tokens itself (duplicated across the 4 cores of a batch,
no collectives needed).

All on-device compute is feature-major ([feature partitions, token free]):
the host supplies x pre-transposed, so the kernel needs zero on-device
transposes. Matmuls run in bf16 with fp32 PSUM accumulation; residual
stream stays fp32. Softmax skips the max-subtraction (|scale*s| < ~8) and
applies the mask multiplicatively after exp; the 1/rowsum is broadcast
across partitions with a K=1 matmul.
"""

import numpy as np
import ml_dtypes

B, S, D, H, CHUNK = 2, 2048, 2048, 16, 1024
HD = D // H          # 128
F = 4 * D            # 8192
T = 512              # query tokens per core
TK = CHUNK           # kv tokens per core
NC = 8
EPS = 1e-5
DB = D // 128        # 16 feature blocks
FB = F // 128        # 64
KC = TK // 128       # 8 key chunks
ATTN_SCALE = 1.0 / float(np.sqrt(HD))

bf16 = ml_dtypes.bfloat16

_CACHE = {}


def _build():
    import concourse.tile as tile
    from concourse import mybir, bacc
    from contextlib import ExitStack

    f32 = mybir.dt.float32
    bfl = mybir.dt.bfloat16
    AF = mybir.ActivationFunctionType
    ALU = mybir.AluOpType

    nc = bacc.Bacc("TRN2", target_bir_lowering=False, debug=False, num_devices=NC)

    xqT = nc.declare_dram_parameter("xqT", [D, T], f32, isOutput=False)
    xkvT = nc.declare_dram_parameter("xkvT", [D, TK], f32, isOutput=False)
    wqkv = nc.declare_dram_parameter("wqkv", [D, 3 * D], bfl, isOutput=False)
    wo = nc.declare_dram_parameter("wo", [D, D], bfl, isOutput=False)
    w1 = nc.declare_dram_parameter("w1", [D, F], bfl, isOutput=False)
    w2 = nc.declare_dram_parameter("w2", [F, D], bfl, isOutput=False)
    maskT = nc.declare_dram_parameter("maskT", [TK, T], bfl, isOutput=False)
    b1T = nc.declare_dram_parameter("b1T", [128, FB], f32, isOutput=False)
    b2T = nc.declare_dram_parameter("b2T", [128, DB], f32, isOutput=False)
    g1T = nc.declare_dram_parameter("g1T", [128, DB], f32, isOutput=False)
    be1T = nc.declare_dram_parameter("be1T", [128, DB], f32, isOutput=False)
    g2T = nc.declare_dram_parameter("g2T", [128, DB], f32, isOutput=False)
    be2T = nc.declare_dram_parameter("be2T", [128, DB], f32, isOutput=False)
    yT = nc.declare_dram_parameter("yT", [D, T], f32, isOutput=True)

    def colblk(t):
        return t.ap().rearrange("(b p) c -> p b c", p=128)

    xqT_v = colblk(xqT)        # [128, 16, 512]
    xkvT_v = colblk(xkvT)      # [128, 16, 1024]
    wqkv_v = colblk(wqkv)      # [128, 16, 6144]
    wo_v = colblk(wo)          # [128, 16, 2048]
    w1_v = colblk(w1)          # [128, 16, 8192]
    w2_v = colblk(w2)          # [128, 64, 2048]
    maskT_v = colblk(maskT)    # [128, 8, 512]
    yT_v = colblk(yT)          # [128, 16, 512]

    with tile.TileContext(nc) as tc, ExitStack() as ctx:
        const = ctx.enter_context(tc.tile_pool(name="const", bufs=1))
        p_rows = ctx.enter_context(tc.tile_pool(name="rows", bufs=1))
        p_bmbr = ctx.enter_context(tc.tile_pool(name="bmbr", bufs=1))
        p_t12 = ctx.enter_context(tc.tile_pool(name="t12", bufs=1))
        p_wcol = ctx.enter_context(tc.tile_pool(name="wcol", bufs=3))
        p_xs = ctx.enter_context(tc.tile_pool(name="xs", bufs=3))
        p_sq = ctx.enter_context(tc.tile_pool(name="sq", bufs=3))
        p_xb = ctx.enter_context(tc.tile_pool(name="xbf", bufs=1))
        ps_mm = ctx.enter_context(tc.tile_pool(name="psmm", bufs=3, space="PSUM"))
        ps_acc = ctx.enter_context(tc.tile_pool(name="psacc", bufs=2, space="PSUM"))
        ps_stat = ctx.enter_context(tc.tile_pool(name="psstat", bufs=2, space="PSUM"))
        ps_lrow = ctx.enter_context(tc.tile_pool(name="pslrow", bufs=1, space="PSUM"))

        ones_col_bf = const.tile([128, 1], bfl)
        nc.vector.memset(ones_col_bf[:], 1.0)
        ones_row_f = const.tile([1, 128], f32)
        nc.vector.memset(ones_row_f[:], 1.0)
        eps_t = const.tile([1, 1], f32)
        nc.vector.memset(eps_t[:], EPS)

        b1t = const.tile([128, FB], f32)
        nc.sync.dma_start(b1t[:], b1T[:, :])
        b2t = const.tile([128, DB], f32)
        nc.sync.dma_start(b2t[:], b2T[:, :])
        g1t = const.tile([128, DB], f32)
        nc.sync.dma_start(g1t[:], g1T[:, :])
        be1t = const.tile([128, DB], f32)
        nc.sync.dma_start(be1t[:], be1T[:, :])
        g2t = const.tile([128, DB], f32)
        nc.sync.dma_start(g2t[:], g2T[:, :])
        be2t = const.tile([128, DB], f32)
        nc.sync.dma_start(be2t[:], be2T[:, :])

        def layer_norm(src_fn, gt, bet, out_write, dram_src=True):
            """src_fn(db) -> fp32 [128, 512] source AP for block db (DRAM if
            dram_src else SBUF). Streams per-db: cast to bf16 (kept), stats
            via ones-matmuls, then normalize from the bf16 copy."""
            NT = T
            xb = p_xb.tile([128, DB, NT], bfl, name="lnxb")
            mean_ps = ps_stat.tile([1, NT], f32, name="stat")
            ss_ps = ps_stat.tile([1, NT], f32, name="stat")
            for db in range(DB):
                src = src_fn(db)
                if dram_src:  # DRAM source: stage in SBUF
                    xs = p_xs.tile([128, NT], f32, name="x32db")
                    nc.sync.dma_start(xs[:], src)
                    src = xs[:]
                nc.scalar.activation(xb[:, db, :], src, AF.Copy)
                nc.tensor.matmul(mean_ps[:], ones_col_bf[:], xb[:, db, :],
                                 start=(db == 0), stop=(db == DB - 1))
                sq = p_sq.tile([128, NT], bfl, name="lnsq")
                nc.scalar.activation(sq[:], xb[:, db, :], AF.Square)
                nc.tensor.matmul(ss_ps[:], ones_col_bf[:], sq[:],
                                 start=(db == 0), stop=(db == DB - 1))
            m_row = p_rows.tile([1, NT], f32, name="m_row")
            nc.vector.tensor_scalar_mul(m_row[:], mean_ps[:], 1.0 / D)
            var = p_rows.tile([1, NT], f32, name="var")
            # var = ss/D - m^2  == (ss/D) - m*m
            nc.vector.tensor_scalar_mul(var[:], ss_ps[:], 1.0 / D)
            m2 = p_rows.tile([1, NT], f32, name="sd")
            nc.vector.tensor_mul(m2[:], m_row[:], m_row[:])
            nc.vector.tensor_sub(var[:], var[:], m2[:])
            sd = p_rows.tile([1, NT], f32, name="sd")
            nc.scalar.activation(sd[:], var[:], AF.Sqrt, bias=eps_t[:])
            rinv = p_rows.tile([1, NT], f32, name="rinv")
            nc.vector.reciprocal(rinv[:], sd[:])
            bm_ps = ps_mm.tile([128, 512], f32, name="mmps")
            nc.tensor.matmul(bm_ps[:, :NT], ones_row_f[:], m_row[:], start=True, stop=True)
            bm = p_bmbr.tile([128, NT], f32, name="bm")
            nc.vector.tensor_copy(bm[:], bm_ps[:, :NT])
            br_ps = ps_mm.tile([128, 512], f32, name="mmps")
            nc.tensor.matmul(br_ps[:, :NT], ones_row_f[:], rinv[:], start=True, stop=True)
            br = p_bmbr.tile([128, NT], f32, name="br")
            nc.vector.tensor_copy(br[:], br_ps[:, :NT])
            for db in range(DB):
                t1 = p_t12.tile([128, NT], f32, name="t1")
                nc.vector.tensor_sub(t1[:], xb[:, db, :], bm[:])
                t2 = p_t12.tile([128, NT], f32, name="t2")
                nc.vector.tensor_mul(t2[:], t1[:], br[:])
                out_write(db, t2, gt[:, db:db + 1], bet[:, db:db + 1])

        with ExitStack() as s_at:
            p_at = s_at.enter_context(tc.tile_pool(name="at", bufs=1))
            # ---- phase B: LN1 + QKV ----
            with ExitStack() as s_qkv:
                p_qkv = s_qkv.enter_context(tc.tile_pool(name="qkv", bufs=1))
                QT = p_qkv.tile([128, H, T], bfl, name="QT")
                KT = p_qkv.tile([128, H, TK], bfl, name="KT")
                V = p_qkv.tile([128, KC, D], bfl, name="V")
                maskS = p_qkv.tile([128, KC, T], bfl, name="maskS")
                nc.sync.dma_start(maskS[:], maskT_v[:, :, :])

                with ExitStack() as s_q:
                    p_q = s_q.enter_context(tc.tile_pool(name="pq", bufs=1))
                    xql = p_q.tile([128, DB, T], bfl, name="xql")

                    def wr_q(db, t2, g, be):
                        nc.vector.tensor_scalar(xql[:, db, :], t2[:], g, be,
                                                op0=ALU.mult, op1=ALU.add)

                    layer_norm(lambda db: xqT_v[:, db, :], g1t, be1t, wr_q)

                    for dq in range(DB):
                        wq = p_wcol.tile([128, DB, 128], bfl, name="wcol")
                        nc.sync.dma_start(wq[:], wqkv_v[:, :, dq * 128:(dq + 1) * 128])
                        ps = ps_mm.tile([128, 512], f32, name="mmps")
                        for db in range(DB):
                            nc.tensor.matmul(ps[:], wq[:, db, :], xql[:, db, :],
                                             start=(db == 0), stop=(db == DB - 1))
                        nc.scalar.activation(QT[:, dq, :], ps[:], AF.Copy)

                with ExitStack() as s_kv:
                    p_kv = s_kv.enter_context(tc.tile_pool(name="pkv", bufs=1))
                    xkvl = p_kv.tile([128, DB, TK], bfl, name="xkvl")
                    for half in range(2):
                        def wr_kv(db, t2, g, be, _h=half):
                            nc.vector.tensor_scalar(xkvl[:, db, _h * T:(_h + 1) * T],
                                                    t2[:], g, be,
                                                    op0=ALU.mult, op1=ALU.add)

                        layer_norm(lambda db, _h=half: xkvT_v[:, db, _h * T:(_h + 1) * T],
                                   g1t, be1t, wr_kv)

                    for dk in range(DB):
                        wk = p_wcol.tile([128, DB, 128], bfl, name="wcol")
                        nc.sync.dma_start(wk[:], wqkv_v[:, :, D + dk * 128: D + (dk + 1) * 128])
                        for tc2 in range(2):
                            ps = ps_mm.tile([128, 512], f32, name="mmps")
                            for db in range(DB):
                                nc.tensor.matmul(ps[:], wk[:, db, :],
                                                 xkvl[:, db, tc2 * 512:(tc2 + 1) * 512],
                                                 start=(db == 0), stop=(db == DB - 1))
                            nc.scalar.activation(KT[:, dk, tc2 * 512:(tc2 + 1) * 512],
                                                 ps[:], AF.Copy)

                    with ExitStack() as s_v:
                        p_wv = s_v.enter_context(tc.tile_pool(name="pwv", bufs=2))
                        for dvs in range(8):  # 256-wide v column slabs
                            wv = p_wv.tile([128, DB, 256], bfl, name="wv")
                            nc.sync.dma_start(
                                wv[:], wqkv_v[:, :, 2 * D + dvs * 256: 2 * D + (dvs + 1) * 256])
                            for tkc in range(KC):
                                ps = ps_mm.tile([128, 512], f32, name="mmps")
                                for db in range(DB):
                                    nc.tensor.matmul(
                                        ps[:, :256], xkvl[:, db, tkc * 128:(tkc + 1) * 128],
                                        wv[:, db, :],
                                        start=(db == 0), stop=(db == DB - 1))
                                nc.scalar.activation(V[:, tkc, dvs * 256:(dvs + 1) * 256],
                                                     ps[:, :256], AF.Copy)

                # ---- phase C: attention ----
                AT = p_at.tile([128, H, T], bfl, name="AT")
                with ExitStack() as s_c:
                    p_pt = s_c.enter_context(tc.tile_pool(name="pt", bufs=3))
                    p_lb = s_c.enter_context(tc.tile_pool(name="lb", bufs=2))
                    for h in range(H):
                        av_ps = ps_acc.tile([128, 512], f32, name="av")
                        l_ps = ps_lrow.tile([1, 512], f32, name="lrow")
                        for kc in range(KC):
                            s_ps = ps_mm.tile([128, 512], f32, name="mmps")
                            nc.tensor.matmul(s_ps[:], KT[:, h, kc * 128:(kc + 1) * 128],
                                             QT[:, h, :], start=True, stop=True)
                            pt = p_pt.tile([128, T], bfl, name="pt")
                            nc.scalar.activation(pt[:], s_ps[:], AF.Exp, scale=ATTN_SCALE)
                            ptm = p_pt.tile([128, T], bfl, name="ptm")
                            nc.vector.tensor_mul(ptm[:], pt[:], maskS[:, kc, :])
                            nc.tensor.matmul(l_ps[:], ones_col_bf[:], ptm[:],
                                             start=(kc == 0), stop=(kc == KC - 1))
                            nc.tensor.matmul(av_ps[:], V[:, kc, h * 128:(h + 1) * 128],
                                             ptm[:],
                                             start=(kc == 0), stop=(kc == KC - 1))
                        linv = p_rows.tile([1, T], f32, name="m_row")
                        nc.vector.reciprocal(linv[:], l_ps[:])
                        bc_ps = ps_mm.tile([128, 512], f32, name="mmps")
                        nc.tensor.matmul(bc_ps[:], ones_row_f[:], linv[:],
                                         start=True, stop=True)
                        lb = p_lb.tile([128, T], f32, name="lbt")
                        nc.vector.tensor_copy(lb[:], bc_ps[:])
                        nc.vector.tensor_mul(AT[:, h, :], av_ps[:], lb[:])

            # ---- phase D: o_proj + residual + LN2 ----
            with ExitStack() as s_e:
                p_e = s_e.enter_context(tc.tile_pool(name="pe", bufs=1))
                x2T = p_e.tile([128, DB, T], f32, name="x2T")
                x2l = p_e.tile([128, DB, T], bfl, name="x2l")
                with ExitStack() as s_d:
                    p_xo = s_d.enter_context(tc.tile_pool(name="pxo", bufs=4))
                    for do in range(DB):
                        woc = p_wcol.tile([128, DB, 128], bfl, name="wcol")
                        nc.sync.dma_start(woc[:], wo_v[:, :, do * 128:(do + 1) * 128])
                        ps = ps_mm.tile([128, 512], f32, name="mmps")
                        for da in range(DB):
                            nc.tensor.matmul(ps[:], woc[:, da, :], AT[:, da, :],
                                             start=(da == 0), stop=(da == DB - 1))
                        xo = p_xo.tile([128, T], f32, name="xo32")
                        nc.sync.dma_start(xo[:], xqT_v[:, do, :])
                        nc.vector.tensor_add(x2T[:, do, :], ps[:], xo[:])

                    def wr_x2(db, t2, g, be):
                        nc.vector.tensor_scalar(x2l[:, db, :], t2[:], g, be,
                                                op0=ALU.mult, op1=ALU.add)

                    layer_norm(lambda db: x2T[:, db, :], g2t, be2t, wr_x2,
                               dram_src=False)

                # ---- phase E: MLP ----
                with ExitStack() as s_mlp:
                    p_h1 = s_mlp.enter_context(tc.tile_pool(name="ph1", bufs=1))
                    p_yst = s_mlp.enter_context(tc.tile_pool(name="yst", bufs=3))
                    h1T = p_h1.tile([128, FB, T], bfl, name="h1T")
                    for f in range(FB):
                        w1c = p_wcol.tile([128, DB, 128], bfl, name="wcol")
                        nc.sync.dma_start(w1c[:], w1_v[:, :, f * 128:(f + 1) * 128])
                        ps = ps_mm.tile([128, 512], f32, name="mmps")
                        for db in range(DB):
                            nc.tensor.matmul(ps[:], w1c[:, db, :], x2l[:, db, :],
                                             start=(db == 0), stop=(db == DB - 1))
                        nc.scalar.activation(h1T[:, f, :], ps[:], AF.Gelu,
                                             bias=b1t[:, f:f + 1])

                    for do in range(DB):
                        ps = ps_acc.tile([128, 512], f32, name="av")
                        for grp in range(4):
                            w2c = p_wcol.tile([128, DB, 128], bfl, name="wcol")
                            nc.sync.dma_start(
                                w2c[:], w2_v[:, grp * DB:(grp + 1) * DB,
                                             do * 128:(do + 1) * 128])
                            for fi in range(DB):
                                fc = grp * DB + fi
                                nc.tensor.matmul(ps[:], w2c[:, fi, :], h1T[:, fc, :],
                                                 start=(fc == 0), stop=(fc == FB - 1))
                        t = p_yst.tile([128, T], f32, name="ycp")
                        nc.scalar.activation(t[:], ps[:], AF.Identity,
                                             bias=b2t[:, do:do + 1])
                        yt = p_yst.tile([128, T], f32, name="yout")
                        nc.vector.tensor_add(yt[:], t[:], x2T[:, do, :])
                        nc.sync.dma_start(yT_v[:, do, :], yt[:])

    nc.compile()
    return nc


def _get_nc():
    if "nc" not in _CACHE:
        _CACHE["nc"] = _build()
    return _CACHE["nc"]


def kernel(x, w_qkv, w_o, w1, b1, w2, b2, g1, be1, g2, be2):
    from concourse.bass_utils import run_bass_kernel_spmd

    nc = _get_nc()

    x = np.asarray(x, np.float32)
    wqkv_b = np.asarray(w_qkv).astype(bf16)
    wo_b = np.asarray(w_o).astype(bf16)
    w1_b = np.asarray(w1).astype(bf16)
    w2_b = np.asarray(w2).astype(bf16)
    b1T = np.ascontiguousarray(np.asarray(b1, np.float32).reshape(FB, 128).T)
    b2T = np.ascontiguousarray(np.asarray(b2, np.float32).reshape(DB, 128).T)
    g1T = np.ascontiguousarray(np.asarray(g1, np.float32).reshape(DB, 128).T)
    be1T = np.ascontiguousarray(np.asarray(be1, np.float32).reshape(DB, 128).T)
    g2T = np.ascontiguousarray(np.asarray(g2, np.float32).reshape(DB, 128).T)
    be2T = np.ascontiguousarray(np.asarray(be2, np.float32).reshape(DB, 128).T)

    # masks: key j allowed iff j <= (s0 + i) % CHUNK; s0 in {0, 512} mod 1024
    i = np.arange(T)
    j = np.arange(TK)
    masks = {}
    for s0m in (0, 512):
        m = (j[:, None] <= (s0m + i)[None, :]).astype(np.float32)
        masks[s0m] = m.astype(bf16)

    xkvT_b = [np.ascontiguousarray(x[b, :TK].T) for b in range(B)]

    in_maps = []
    for c in range(NC):
        b = c // 4
        s0 = (c % 4) * T
        in_maps.append({
            "xqT": np.ascontiguousarray(x[b, s0:s0 + T].T),
            "xkvT": xkvT_b[b],
            "wqkv": wqkv_b, "wo": wo_b, "w1": w1_b, "w2": w2_b,
            "maskT": masks[s0 % CHUNK],
            "b1T": b1T, "b2T": b2T, "g1T": g1T, "be1T": be1T,
            "g2T": g2T, "be2T": be2T,
        })

    res = run_bass_kernel_spmd(nc, in_maps, list(range(NC)))

    out = np.empty((B, S, D), np.float32)
    for c in range(NC):
        b = c // 4
        s0 = (c % 4) * T
        out[b, s0:s0 + T] = res.results[c]["yT"].T
    return out


# revision 10
# speedup vs baseline: 1.1077x; 1.1077x over previous
"""Fused transformer layer (LN->attn->LN->MLP, residuals) on 8 NeuronCores.

Sharding: pure sequence/data parallel - core c handles batch c//4, query
tokens (c%4)*512..+512. The reference mask allows key j iff j <= q%1024, so
only keys 0..1023 of each batch are ever attended; each core computes k/v
for those 1024 tokens itself (duplicated across the 4 cores of a batch,
no collectives needed).

All on-device compute is feature-major ([feature partitions, token free]):
the host supplies x pre-transposed, so the kernel needs zero on-device
transposes. Matmuls run in bf16 with fp32 PSUM accumulation; residual
stream stays fp32. Softmax skips the max-subtraction (|scale*s| < ~8) and
applies the mask multiplicatively after exp; the 1/rowsum is broadcast
across partitions with a K=1 matmul.
"""

import numpy as np
import ml_dtypes

B, S, D, H, CHUNK = 2, 2048, 2048, 16, 1024
HD = D // H          # 128
F = 4 * D            # 8192
T = 512              # query tokens per core
TK = CHUNK           # kv tokens per core
NC = 8
EPS = 1e-5
DB = D // 128        # 16 feature blocks
FB = F // 128        # 64
KC = TK // 128       # 8 key chunks
ATTN_SCALE = 1.0 / float(np.sqrt(HD))

bf16 = ml_dtypes.bfloat16

_CACHE = {}


def _build():
    import concourse.tile as tile
    from concourse import mybir, bacc
    from contextlib import ExitStack

    f32 = mybir.dt.float32
    bfl = mybir.dt.bfloat16
    AF = mybir.ActivationFunctionType
    ALU = mybir.AluOpType

    nc = bacc.Bacc("TRN2", target_bir_lowering=False, debug=False, num_devices=NC)

    xqT = nc.declare_dram_parameter("xqT", [D, T], f32, isOutput=False)
    xkvT = nc.declare_dram_parameter("xkvT", [D, TK], f32, isOutput=False)
    wqkv = nc.declare_dram_parameter("wqkv", [D, 3 * D], bfl, isOutput=False)
    wo = nc.declare_dram_parameter("wo", [D, D], bfl, isOutput=False)
    w1 = nc.declare_dram_parameter("w1", [D, F], bfl, isOutput=False)
    w2 = nc.declare_dram_parameter("w2", [F, D], bfl, isOutput=False)
    maskT = nc.declare_dram_parameter("maskT", [TK, T], bfl, isOutput=False)
    b1T = nc.declare_dram_parameter("b1T", [128, FB], f32, isOutput=False)
    b2T = nc.declare_dram_parameter("b2T", [128, DB], f32, isOutput=False)
    g1T = nc.declare_dram_parameter("g1T", [128, DB], f32, isOutput=False)
    be1T = nc.declare_dram_parameter("be1T", [128, DB], f32, isOutput=False)
    g2T = nc.declare_dram_parameter("g2T", [128, DB], f32, isOutput=False)
    be2T = nc.declare_dram_parameter("be2T", [128, DB], f32, isOutput=False)
    yT = nc.declare_dram_parameter("yT", [D, T], f32, isOutput=True)

    def colblk(t):
        return t.ap().rearrange("(b p) c -> p b c", p=128)

    xqT_v = colblk(xqT)        # [128, 16, 512]
    xkvT_v = colblk(xkvT)      # [128, 16, 1024]
    wqkv_v = colblk(wqkv)      # [128, 16, 6144]
    wo_v = colblk(wo)          # [128, 16, 2048]
    w1_v = colblk(w1)          # [128, 16, 8192]
    w2_v = colblk(w2)          # [128, 64, 2048]
    maskT_v = colblk(maskT)    # [128, 8, 512]
    yT_v = colblk(yT)          # [128, 16, 512]

    with tile.TileContext(nc) as tc, ExitStack() as ctx:
        const = ctx.enter_context(tc.tile_pool(name="const", bufs=1))
        p_rows = ctx.enter_context(tc.tile_pool(name="rows", bufs=1))
        p_bmbr = ctx.enter_context(tc.tile_pool(name="bmbr", bufs=1))
        p_t12 = ctx.enter_context(tc.tile_pool(name="t12", bufs=1))
        p_wcol = ctx.enter_context(tc.tile_pool(name="wcol", bufs=3))
        p_xs = ctx.enter_context(tc.tile_pool(name="xs", bufs=3))
        p_sq = ctx.enter_context(tc.tile_pool(name="sq", bufs=3))
        p_xb = ctx.enter_context(tc.tile_pool(name="xbf", bufs=16))
        ps_mm = ctx.enter_context(tc.tile_pool(name="psmm", bufs=3, space="PSUM"))
        ps_acc = ctx.enter_context(tc.tile_pool(name="psacc", bufs=2, space="PSUM"))
        ps_stat = ctx.enter_context(tc.tile_pool(name="psstat", bufs=2, space="PSUM"))
        ps_lrow = ctx.enter_context(tc.tile_pool(name="pslrow", bufs=1, space="PSUM"))

        ones_col_bf = const.tile([128, 1], bfl)
        nc.vector.memset(ones_col_bf[:], 1.0)
        ones_row_f = const.tile([1, 128], f32)
        nc.vector.memset(ones_row_f[:], 1.0)
        eps_t = const.tile([1, 1], f32)
        nc.vector.memset(eps_t[:], EPS)

        b1t = const.tile([128, FB], f32)
        nc.sync.dma_start(b1t[:], b1T[:, :])
        b2t = const.tile([128, DB], f32)
        nc.sync.dma_start(b2t[:], b2T[:, :])
        g1t = const.tile([128, DB], f32)
        nc.sync.dma_start(g1t[:], g1T[:, :])
        be1t = const.tile([128, DB], f32)
        nc.sync.dma_start(be1t[:], be1T[:, :])
        g2t = const.tile([128, DB], f32)
        nc.sync.dma_start(g2t[:], g2T[:, :])
        be2t = const.tile([128, DB], f32)
        nc.sync.dma_start(be2t[:], be2T[:, :])

        def layer_norm(src_fn, gt, bet, dst_fn, dram_src=True):
            """src_fn(db) -> fp32 [128, 512] source AP for block db (DRAM if
            dram_src else SBUF). dst_fn(db) -> bf16 [128, 512] output AP.
            Streams per-db: cast to bf16 (kept), stats via ones-matmuls,
            then normalize from the bf16 copy; affine applied on ACT."""
            NT = T
            xbs = []
            mean_ps = ps_stat.tile([1, NT], f32, name="stat")
            ss_ps = ps_stat.tile([1, NT], f32, name="stat")
            for db in range(DB):
                src = src_fn(db)
                if dram_src:  # DRAM source: stage in SBUF
                    xs = p_xs.tile([128, NT], f32, name="x32db")
                    nc.sync.dma_start(xs[:], src)
                    src = xs[:]
                xb = p_xb.tile([128, NT], bfl, name="lnxb")
                xbs.append(xb)
                nc.scalar.activation(xb[:], src, AF.Copy)
                nc.tensor.matmul(mean_ps[:], ones_col_bf[:], xb[:],
                                 start=(db == 0), stop=(db == DB - 1))
                sq = p_sq.tile([128, NT], bfl, name="lnsq")
                nc.scalar.activation(sq[:], xb[:], AF.Square)
                nc.tensor.matmul(ss_ps[:], ones_col_bf[:], sq[:],
                                 start=(db == 0), stop=(db == DB - 1))
            m_row = p_rows.tile([1, NT], f32, name="m_row")
            nc.vector.tensor_scalar_mul(m_row[:], mean_ps[:], 1.0 / D)
            var = p_rows.tile([1, NT], f32, name="var")
            # var = ss/D - m^2  == (ss/D) - m*m
            nc.vector.tensor_scalar_mul(var[:], ss_ps[:], 1.0 / D)
            m2 = p_rows.tile([1, NT], f32, name="sd")
            nc.vector.tensor_mul(m2[:], m_row[:], m_row[:])
            nc.vector.tensor_sub(var[:], var[:], m2[:])
            sd = p_rows.tile([1, NT], f32, name="sd")
            nc.scalar.activation(sd[:], var[:], AF.Sqrt, bias=eps_t[:])
            rinv = p_rows.tile([1, NT], f32, name="rinv")
            nc.vector.reciprocal_approx_fast(rinv[:], sd[:])
            bm_ps = ps_mm.tile([128, 512], f32, name="mmps")
            nc.tensor.matmul(bm_ps[:, :NT], ones_row_f[:], m_row[:], start=True, stop=True)
            bm = p_bmbr.tile([128, NT], f32, name="bm")
            nc.vector.tensor_copy(bm[:], bm_ps[:, :NT])
            br_ps = ps_mm.tile([128, 512], f32, name="mmps")
            nc.tensor.matmul(br_ps[:, :NT], ones_row_f[:], rinv[:], start=True, stop=True)
            br = p_bmbr.tile([128, NT], f32, name="br")
            nc.vector.tensor_copy(br[:], br_ps[:, :NT])
            for db in range(DB):
                t1 = p_t12.tile([128, NT], f32, name="t1")
                nc.vector.tensor_sub(t1[:], xbs[db][:], bm[:])
                t2 = p_t12.tile([128, NT], f32, name="t2")
                nc.vector.tensor_mul(t2[:], t1[:], br[:])
                nc.scalar.activation(dst_fn(db), t2[:], AF.Identity,
                                     bias=bet[:, db:db + 1], scale=gt[:, db:db + 1])

        with ExitStack() as s_at:
            p_at = s_at.enter_context(tc.tile_pool(name="at", bufs=1))
            # ---- phase B: LN1 + QKV ----
            with ExitStack() as s_qkv:
                p_qkv = s_qkv.enter_context(tc.tile_pool(name="qkv", bufs=1))
                QT = p_qkv.tile([128, H, T], bfl, name="QT")
                KT = p_qkv.tile([128, H, TK], bfl, name="KT")
                V = p_qkv.tile([128, KC, D], bfl, name="V")
                maskS = p_qkv.tile([128, KC, T], bfl, name="maskS")
                nc.sync.dma_start(maskS[:], maskT_v[:, :, :])

                with ExitStack() as s_q:
                    p_q = s_q.enter_context(tc.tile_pool(name="pq", bufs=16))
                    xql = [p_q.tile([128, T], bfl, name="xql") for _ in range(DB)]

                    layer_norm(lambda db: xqT_v[:, db, :], g1t, be1t,
                               lambda db: xql[db][:])

                    for dq in range(DB):
                        wq = p_wcol.tile([128, DB, 128], bfl, name="wcol")
                        nc.sync.dma_start(wq[:], wqkv_v[:, :, dq * 128:(dq + 1) * 128])
                        ps = ps_mm.tile([128, 512], f32, name="mmps")
                        for db in range(DB):
                            nc.tensor.matmul(ps[:], wq[:, db, :], xql[db][:],
                                             start=(db == 0), stop=(db == DB - 1))
                        nc.scalar.activation(QT[:, dq, :], ps[:], AF.Copy)

                with ExitStack() as s_kv:
                    p_kv = s_kv.enter_context(tc.tile_pool(name="pkv", bufs=32))
                    xkvl = [[None] * DB for _ in range(2)]
                    for half in range(2):
                        for db in range(DB):
                            xkvl[half][db] = p_kv.tile([128, T], bfl, name="xkvl")

                    def emit_k_half(tc2):
                        for dk in range(DB):
                            wk = p_wcol.tile([128, DB, 128], bfl, name="wcol")
                            nc.sync.dma_start(
                                wk[:], wqkv_v[:, :, D + dk * 128: D + (dk + 1) * 128])
                            ps = ps_mm.tile([128, 512], f32, name="mmps")
                            for db in range(DB):
                                nc.tensor.matmul(ps[:], wk[:, db, :], xkvl[tc2][db][:],
                                                 start=(db == 0), stop=(db == DB - 1))
                            nc.scalar.activation(KT[:, dk, tc2 * 512:(tc2 + 1) * 512],
                                                 ps[:], AF.Copy)

                    for half in range(2):
                        layer_norm(
                            lambda db, _h=half: xkvT_v[:, db, _h * T:(_h + 1) * T],
                            g1t, be1t,
                            lambda db, _h=half: xkvl[_h][db][:])
                        emit_k_half(half)

                    with ExitStack() as s_v:
                        p_wv = s_v.enter_context(tc.tile_pool(name="pwv", bufs=2))
                        for dvs in range(8):  # 256-wide v column slabs
                            wv = p_wv.tile([128, DB, 256], bfl, name="wv")
                            nc.sync.dma_start(
                                wv[:], wqkv_v[:, :, 2 * D + dvs * 256: 2 * D + (dvs + 1) * 256])
                            for tkc in range(KC):
                                ps = ps_mm.tile([128, 512], f32, name="mmps")
                                for db in range(DB):
                                    nc.tensor.matmul(
                                        ps[:, :256],
                                        xkvl[tkc // 4][db][:, (tkc % 4) * 128:(tkc % 4 + 1) * 128],
                                        wv[:, db, :],
                                        start=(db == 0), stop=(db == DB - 1))
                                nc.scalar.activation(V[:, tkc, dvs * 256:(dvs + 1) * 256],
                                                     ps[:, :256], AF.Copy)

                # ---- phase C: attention ----
                AT = p_at.tile([128, H, T], bfl, name="AT")
                with ExitStack() as s_c:
                    p_pt = s_c.enter_context(tc.tile_pool(name="pt", bufs=3))
                    p_lb = s_c.enter_context(tc.tile_pool(name="lb", bufs=2))
                    for h in range(H):
                        av_ps = ps_acc.tile([128, 512], f32, name="av")
                        l_ps = ps_lrow.tile([1, 512], f32, name="lrow")
                        for kc in range(KC):
                            s_ps = ps_mm.tile([128, 512], f32, name="mmps")
                            nc.tensor.matmul(s_ps[:], KT[:, h, kc * 128:(kc + 1) * 128],
                                             QT[:, h, :], start=True, stop=True)
                            pt = p_pt.tile([128, T], bfl, name="pt")
                            nc.scalar.activation(pt[:], s_ps[:], AF.Exp, scale=ATTN_SCALE)
                            ptm = p_pt.tile([128, T], bfl, name="ptm")
                            nc.vector.tensor_mul(ptm[:], pt[:], maskS[:, kc, :])
                            nc.tensor.matmul(l_ps[:], ones_col_bf[:], ptm[:],
                                             start=(kc == 0), stop=(kc == KC - 1))
                            nc.tensor.matmul(av_ps[:], V[:, kc, h * 128:(h + 1) * 128],
                                             ptm[:],
                                             start=(kc == 0), stop=(kc == KC - 1))
                        lrow = p_rows.tile([1, T], f32, name="m_row")
                        nc.vector.tensor_copy(lrow[:], l_ps[:])
                        bc_ps = ps_mm.tile([128, 512], f32, name="mmps")
                        nc.tensor.matmul(bc_ps[:], ones_row_f[:], lrow[:],
                                         start=True, stop=True)
                        lb = p_lb.tile([128, T], f32, name="lbt")
                        nc.vector.reciprocal_approx_fast(lb[:], bc_ps[:])
                        nc.vector.tensor_mul(AT[:, h, :], av_ps[:], lb[:])

            # ---- phase D: o_proj + residual + LN2 ----
            with ExitStack() as s_e:
                p_e = s_e.enter_context(tc.tile_pool(name="pe", bufs=1))
                p_e16 = s_e.enter_context(tc.tile_pool(name="pe16", bufs=16))
                x2T = p_e.tile([128, DB, T], f32, name="x2T")
                x2l = [p_e16.tile([128, T], bfl, name="x2l") for _ in range(DB)]
                with ExitStack() as s_d:
                    p_xo = s_d.enter_context(tc.tile_pool(name="pxo", bufs=4))
                    for do in range(DB):
                        woc = p_wcol.tile([128, DB, 128], bfl, name="wcol")
                        nc.sync.dma_start(woc[:], wo_v[:, :, do * 128:(do + 1) * 128])
                        ps = ps_mm.tile([128, 512], f32, name="mmps")
                        for da in range(DB):
                            nc.tensor.matmul(ps[:], woc[:, da, :], AT[:, da, :],
                                             start=(da == 0), stop=(da == DB - 1))
                        xo = p_xo.tile([128, T], f32, name="xo32")
                        nc.sync.dma_start(xo[:], xqT_v[:, do, :])
                        nc.vector.tensor_add(x2T[:, do, :], ps[:], xo[:])

                    layer_norm(lambda db: x2T[:, db, :], g2t, be2t,
                               lambda db: x2l[db][:], dram_src=False)

                # ---- phase E: MLP ----
                with ExitStack() as s_mlp:
                    p_h1 = s_mlp.enter_context(tc.tile_pool(name="ph1", bufs=1))
                    p_yst = s_mlp.enter_context(tc.tile_pool(name="yst", bufs=3))
                    h1T = p_h1.tile([128, FB, T], bfl, name="h1T")
                    for f in range(FB):
                        w1c = p_wcol.tile([128, DB, 128], bfl, name="wcol")
                        nc.sync.dma_start(w1c[:], w1_v[:, :, f * 128:(f + 1) * 128])
                        ps = ps_mm.tile([128, 512], f32, name="mmps")
                        for db in range(DB):
                            nc.tensor.matmul(ps[:], w1c[:, db, :], x2l[db][:],
                                             start=(db == 0), stop=(db == DB - 1))
                        nc.scalar.activation(h1T[:, f, :], ps[:], AF.Gelu,
                                             bias=b1t[:, f:f + 1])

                    for do in range(DB):
                        ps = ps_acc.tile([128, 512], f32, name="av")
                        for grp in range(4):
                            w2c = p_wcol.tile([128, DB, 128], bfl, name="wcol")
                            nc.sync.dma_start(
                                w2c[:], w2_v[:, grp * DB:(grp + 1) * DB,
                                             do * 128:(do + 1) * 128])
                            for fi in range(DB):
                                fc = grp * DB + fi
                                nc.tensor.matmul(ps[:], w2c[:, fi, :], h1T[:, fc, :],
                                                 start=(fc == 0), stop=(fc == FB - 1))
                        t = p_yst.tile([128, T], f32, name="ycp")
                        nc.scalar.activation(t[:], ps[:], AF.Identity,
                                             bias=b2t[:, do:do + 1])
                        yt = p_yst.tile([128, T], f32, name="yout")
                        nc.vector.tensor_add(yt[:], t[:], x2T[:, do, :])
                        nc.sync.dma_start(yT_v[:, do, :], yt[:])

    nc.compile()
    return nc


def _get_nc():
    if "nc" not in _CACHE:
        _CACHE["nc"] = _build()
    return _CACHE["nc"]


def kernel(x, w_qkv, w_o, w1, b1, w2, b2, g1, be1, g2, be2):
    from concourse.bass_utils import run_bass_kernel_spmd

    nc = _get_nc()

    x = np.asarray(x, np.float32)
    wqkv_b = np.asarray(w_qkv).astype(bf16)
    wo_b = np.asarray(w_o).astype(bf16)
    w1_b = np.asarray(w1).astype(bf16)
    w2_b = np.asarray(w2).astype(bf16)
    b1T = np.ascontiguousarray(np.asarray(b1, np.float32).reshape(FB, 128).T)
    b2T = np.ascontiguousarray(np.asarray(b2, np.float32).reshape(DB, 128).T)
    g1T = np.ascontiguousarray(np.asarray(g1, np.float32).reshape(DB, 128).T)
    be1T = np.ascontiguousarray(np.asarray(be1, np.float32).reshape(DB, 128).T)
    g2T = np.ascontiguousarray(np.asarray(g2, np.float32).reshape(DB, 128).T)
    be2T = np.ascontiguousarray(np.asarray(be2, np.float32).reshape(DB, 128).T)

    # masks: key j allowed iff j <= (s0 + i) % CHUNK; s0 in {0, 512} mod 1024
    i = np.arange(T)
    j = np.arange(TK)
    masks = {}
    for s0m in (0, 512):
        m = (j[:, None] <= (s0m + i)[None, :]).astype(np.float32)
        masks[s0m] = m.astype(bf16)

    xkvT_b = [np.ascontiguousarray(x[b, :TK].T) for b in range(B)]

    in_maps = []
    for c in range(NC):
        b = c // 4
        s0 = (c % 4) * T
        in_maps.append({
            "xqT": np.ascontiguousarray(x[b, s0:s0 + T].T),
            "xkvT": xkvT_b[b],
            "wqkv": wqkv_b, "wo": wo_b, "w1": w1_b, "w2": w2_b,
            "maskT": masks[s0 % CHUNK],
            "b1T": b1T, "b2T": b2T, "g1T": g1T, "be1T": be1T,
            "g2T": g2T, "be2T": be2T,
        })

    res = run_bass_kernel_spmd(nc, in_maps, list(range(NC)))

    out = np.empty((B, S, D), np.float32)
    for c in range(NC):
        b = c // 4
        s0 = (c % 4) * T
        out[b, s0:s0 + T] = res.results[c]["yT"].T
    return out


# revision 14
# speedup vs baseline: 1.1208x; 1.0119x over previous
"""Fused transformer layer (LN->attn->LN->MLP, residuals) on 8 NeuronCores.

Sharding: pure sequence/data parallel - core c handles batch c//4, query
tokens (c%4)*512..+512. The reference mask allows key j iff j <= q%1024, so
only keys 0..1023 of each batch are ever attended; each core computes k/v
for those 1024 tokens itself (duplicated across the 4 cores of a batch,
no collectives needed).

All on-device compute is feature-major ([feature partitions, token free]):
the host supplies x pre-transposed, so the kernel needs zero on-device
transposes. Matmuls run in bf16 with fp32 PSUM accumulation; residual
stream stays fp32. Softmax skips the max-subtraction (|scale*s| < ~8) and
applies the mask multiplicatively after exp; the 1/rowsum is broadcast
across partitions with a K=1 matmul.
"""

import numpy as np
import ml_dtypes

B, S, D, H, CHUNK = 2, 2048, 2048, 16, 1024
HD = D // H          # 128
F = 4 * D            # 8192
T = 512              # query tokens per core
TK = CHUNK           # kv tokens per core
NC = 8
EPS = 1e-5
DB = D // 128        # 16 feature blocks
FB = F // 128        # 64
KC = TK // 128       # 8 key chunks
ATTN_SCALE = 1.0 / float(np.sqrt(HD))

bf16 = ml_dtypes.bfloat16

_CACHE = {}


def _build():
    import concourse.tile as tile
    from concourse import mybir, bacc
    from contextlib import ExitStack

    f32 = mybir.dt.float32
    bfl = mybir.dt.bfloat16
    AF = mybir.ActivationFunctionType
    ALU = mybir.AluOpType

    nc = bacc.Bacc("TRN2", target_bir_lowering=False, debug=False, num_devices=NC)

    xqT = nc.declare_dram_parameter("xqT", [D, T], f32, isOutput=False)
    xkvT = nc.declare_dram_parameter("xkvT", [D, TK], f32, isOutput=False)
    wq = nc.declare_dram_parameter("wq", [D, D], bfl, isOutput=False)
    wk_sh = nc.declare_dram_parameter("wk_sh", [D, 512], bfl, isOutput=False)
    wv_sh = nc.declare_dram_parameter("wv_sh", [D, 512], bfl, isOutput=False)
    wo = nc.declare_dram_parameter("wo", [D, D], bfl, isOutput=False)
    w1 = nc.declare_dram_parameter("w1", [D, F], bfl, isOutput=False)
    w2 = nc.declare_dram_parameter("w2", [F, D], bfl, isOutput=False)
    maskT = nc.declare_dram_parameter("maskT", [TK, T], bfl, isOutput=False)
    b1T = nc.declare_dram_parameter("b1T", [128, FB], f32, isOutput=False)
    b2T = nc.declare_dram_parameter("b2T", [128, DB], f32, isOutput=False)
    g1T = nc.declare_dram_parameter("g1T", [128, DB], f32, isOutput=False)
    be1T = nc.declare_dram_parameter("be1T", [128, DB], f32, isOutput=False)
    g2T = nc.declare_dram_parameter("g2T", [128, DB], f32, isOutput=False)
    be2T = nc.declare_dram_parameter("be2T", [128, DB], f32, isOutput=False)
    yT = nc.declare_dram_parameter("yT", [D, T], f32, isOutput=True)

    def colblk(t):
        return t.ap().rearrange("(b p) c -> p b c", p=128)

    xqT_v = colblk(xqT)        # [128, 16, 512]
    xkvT_v = colblk(xkvT)      # [128, 16, 1024]
    wq_v = colblk(wq)          # [128, 16, 2048]
    wk_v = colblk(wk_sh)       # [128, 16, 512]
    wv_v = colblk(wv_sh)       # [128, 16, 512]
    wo_v = colblk(wo)          # [128, 16, 2048]
    w1_v = colblk(w1)          # [128, 16, 8192]
    w2_v = colblk(w2)          # [128, 64, 2048]
    maskT_v = colblk(maskT)    # [128, 8, 512]
    yT_v = colblk(yT)          # [128, 16, 512]

    with tile.TileContext(nc) as tc, ExitStack() as ctx:
        const = ctx.enter_context(tc.tile_pool(name="const", bufs=1))
        p_rows = ctx.enter_context(tc.tile_pool(name="rows", bufs=1))
        p_bmbr = ctx.enter_context(tc.tile_pool(name="bmbr", bufs=1))
        p_t12 = ctx.enter_context(tc.tile_pool(name="t12", bufs=1))
        p_wcol = ctx.enter_context(tc.tile_pool(name="wcol", bufs=3))
        p_xs = ctx.enter_context(tc.tile_pool(name="xs", bufs=3))
        p_sq = ctx.enter_context(tc.tile_pool(name="sq", bufs=3))
        p_xb = ctx.enter_context(tc.tile_pool(name="xbf", bufs=16))
        ps_mm = ctx.enter_context(tc.tile_pool(name="psmm", bufs=3, space="PSUM"))
        ps_acc = ctx.enter_context(tc.tile_pool(name="psacc", bufs=2, space="PSUM"))
        ps_stat = ctx.enter_context(tc.tile_pool(name="psstat", bufs=2, space="PSUM"))
        ps_lrow = ctx.enter_context(tc.tile_pool(name="pslrow", bufs=1, space="PSUM"))

        ones_col_bf = const.tile([128, 1], bfl)
        nc.vector.memset(ones_col_bf[:], 1.0)
        ones_row_f = const.tile([1, 128], f32)
        nc.vector.memset(ones_row_f[:], 1.0)
        eps_t = const.tile([1, 1], f32)
        nc.vector.memset(eps_t[:], EPS)

        b1t = const.tile([128, FB], f32)
        nc.sync.dma_start(b1t[:], b1T[:, :])
        b2t = const.tile([128, DB], f32)
        nc.sync.dma_start(b2t[:], b2T[:, :])
        g1t = const.tile([128, DB], f32)
        nc.sync.dma_start(g1t[:], g1T[:, :])
        be1t = const.tile([128, DB], f32)
        nc.sync.dma_start(be1t[:], be1T[:, :])
        g2t = const.tile([128, DB], f32)
        nc.sync.dma_start(g2t[:], g2T[:, :])
        be2t = const.tile([128, DB], f32)
        nc.sync.dma_start(be2t[:], be2T[:, :])

        def layer_norm(src_fn, gt, bet, dst_fn, dram_src=True):
            """src_fn(db) -> fp32 [128, 512] source AP for block db (DRAM if
            dram_src else SBUF). dst_fn(db) -> bf16 [128, 512] output AP.
            Streams per-db: cast to bf16 (kept), stats via ones-matmuls,
            then normalize from the bf16 copy; affine applied on ACT."""
            NT = T
            xbs = []
            mean_ps = ps_stat.tile([1, NT], f32, name="stat")
            ss_ps = ps_stat.tile([1, NT], f32, name="stat")
            for db in range(DB):
                src = src_fn(db)
                if dram_src:  # DRAM source: stage in SBUF
                    xs = p_xs.tile([128, NT], f32, name="x32db")
                    nc.sync.dma_start(xs[:], src)
                    src = xs[:]
                xb = p_xb.tile([128, NT], bfl, name="lnxb")
                xbs.append(xb)
                nc.scalar.activation(xb[:], src, AF.Copy)
                nc.tensor.matmul(mean_ps[:], ones_col_bf[:], xb[:],
                                 start=(db == 0), stop=(db == DB - 1))
                sq = p_sq.tile([128, NT], bfl, name="lnsq")
                nc.scalar.activation(sq[:], xb[:], AF.Square)
                nc.tensor.matmul(ss_ps[:], ones_col_bf[:], sq[:],
                                 start=(db == 0), stop=(db == DB - 1))
            m_row = p_rows.tile([1, NT], f32, name="m_row")
            nc.vector.tensor_scalar_mul(m_row[:], mean_ps[:], 1.0 / D)
            var = p_rows.tile([1, NT], f32, name="var")
            # var = ss/D - m^2  == (ss/D) - m*m
            nc.vector.tensor_scalar_mul(var[:], ss_ps[:], 1.0 / D)
            m2 = p_rows.tile([1, NT], f32, name="sd")
            nc.vector.tensor_mul(m2[:], m_row[:], m_row[:])
            nc.vector.tensor_sub(var[:], var[:], m2[:])
            sd = p_rows.tile([1, NT], f32, name="sd")
            nc.scalar.activation(sd[:], var[:], AF.Sqrt, bias=eps_t[:])
            rinv = p_rows.tile([1, NT], f32, name="rinv")
            nc.vector.reciprocal_approx_fast(rinv[:], sd[:])
            bm_ps = ps_mm.tile([128, 512], f32, name="mmps")
            nc.tensor.matmul(bm_ps[:, :NT], ones_row_f[:], m_row[:], start=True, stop=True)
            bm = p_bmbr.tile([128, NT], f32, name="bm")
            nc.vector.tensor_copy(bm[:], bm_ps[:, :NT])
            br_ps = ps_mm.tile([128, 512], f32, name="mmps")
            nc.tensor.matmul(br_ps[:, :NT], ones_row_f[:], rinv[:], start=True, stop=True)
            br = p_bmbr.tile([128, NT], f32, name="br")
            nc.vector.tensor_copy(br[:], br_ps[:, :NT])
            for db in range(DB):
                t1 = p_t12.tile([128, NT], f32, name="t1")
                nc.vector.tensor_sub(t1[:], xbs[db][:], bm[:])
                t2 = p_t12.tile([128, NT], f32, name="t2")
                nc.vector.tensor_mul(t2[:], t1[:], br[:])
                nc.scalar.activation(dst_fn(db), t2[:], AF.Identity,
                                     bias=bet[:, db:db + 1], scale=gt[:, db:db + 1])

        with ExitStack() as s_at:
            p_at = s_at.enter_context(tc.tile_pool(name="at", bufs=1))
            p_dram = s_at.enter_context(tc.tile_pool(name="dramb", bufs=1, space="DRAM"))
            k_in = p_dram.tile([4, 128, TK], bfl, name="k_in")
            k_out = p_dram.tile([4, 4, 128, TK], bfl, name="k_out")
            v_in = p_dram.tile([KC, 128, 512], bfl, name="v_in")
            v_out = p_dram.tile([4, KC, 128, 512], bfl, name="v_out")
            RG = [[0, 1, 2, 3], [4, 5, 6, 7]]

            # ---- phase B: LN1 + QKV (K/V sharded 4-way, AllGather) ----
            with ExitStack() as s_qkv:
                p_qkv = s_qkv.enter_context(tc.tile_pool(name="qkv", bufs=1))
                p_st = s_qkv.enter_context(tc.tile_pool(name="stage", bufs=1))
                QT = p_qkv.tile([128, H, T], bfl, name="QT")

                with ExitStack() as s_kv:
                    p_kv = s_kv.enter_context(tc.tile_pool(name="pkv", bufs=32))
                    p_wvs = s_kv.enter_context(tc.tile_pool(name="pwvs", bufs=2))
                    xkvl = [[None] * DB for _ in range(2)]
                    for half in range(2):
                        for db in range(DB):
                            xkvl[half][db] = p_kv.tile([128, T], bfl, name="xkvl")
                    kstage = [p_st.tile([128, TK], bfl, name=f"kst{i}") for i in range(4)]
                    vstage = [p_st.tile([128, 512], bfl, name=f"vst{i}") for i in range(KC)]

                    def emit_kv_half(tc2):
                        # K shard: 4 dk blocks, this token half
                        for dkl in range(4):
                            wk = p_wcol.tile([128, DB, 128], bfl, name="wcol")
                            nc.sync.dma_start(wk[:], wk_v[:, :, dkl * 128:(dkl + 1) * 128])
                            ps = ps_mm.tile([128, 512], f32, name="mmps")
                            for db in range(DB):
                                nc.tensor.matmul(ps[:], wk[:, db, :], xkvl[tc2][db][:],
                                                 start=(db == 0), stop=(db == DB - 1))
                            nc.scalar.activation(kstage[dkl][:, tc2 * 512:(tc2 + 1) * 512],
                                                 ps[:], AF.Copy)
                        # V shard: 2 x 256-wide column slabs, token chunks of this half
                        for vs in range(2):
                            wv = p_wvs.tile([128, DB, 256], bfl, name="wv")
                            nc.sync.dma_start(wv[:], wv_v[:, :, vs * 256:(vs + 1) * 256])
                            for tl in range(4):
                                tkc = tc2 * 4 + tl
                                ps = ps_mm.tile([128, 512], f32, name="mmps")
                                for db in range(DB):
                                    nc.tensor.matmul(
                                        ps[:, :256],
                                        xkvl[tc2][db][:, tl * 128:(tl + 1) * 128],
                                        wv[:, db, :],
                                        start=(db == 0), stop=(db == DB - 1))
                                nc.scalar.activation(vstage[tkc][:, vs * 256:(vs + 1) * 256],
                                                     ps[:, :256], AF.Copy)

                    for half in range(2):
                        layer_norm(
                            lambda db, _h=half: xkvT_v[:, db, _h * T:(_h + 1) * T],
                            g1t, be1t,
                            lambda db, _h=half: xkvl[_h][db][:])
                        emit_kv_half(half)

                    for dkl in range(4):
                        nc.sync.dma_start(k_in[dkl, :, :], kstage[dkl][:])
                    for tkc in range(KC):
                        nc.sync.dma_start(v_in[tkc, :, :], vstage[tkc][:])

                nc.gpsimd.collective_compute(
                    "AllGather", ALU.bypass, replica_groups=RG,
                    ins=[k_in.opt()], outs=[k_out.opt()])
                nc.gpsimd.collective_compute(
                    "AllGather", ALU.bypass, replica_groups=RG,
                    ins=[v_in.opt()], outs=[v_out.opt()])

                # Q projection (overlaps the collectives)
                with ExitStack() as s_q:
                    p_q = s_q.enter_context(tc.tile_pool(name="pq", bufs=16))
                    xql = [p_q.tile([128, T], bfl, name="xql") for _ in range(DB)]

                    layer_norm(lambda db: xqT_v[:, db, :], g1t, be1t,
                               lambda db: xql[db][:])

                    for dq in range(DB):
                        wqc = p_wcol.tile([128, DB, 128], bfl, name="wcol")
                        nc.sync.dma_start(wqc[:], wq_v[:, :, dq * 128:(dq + 1) * 128])
                        ps = ps_mm.tile([128, 512], f32, name="mmps")
                        for db in range(DB):
                            nc.tensor.matmul(ps[:], wqc[:, db, :], xql[db][:],
                                             start=(db == 0), stop=(db == DB - 1))
                        nc.scalar.activation(QT[:, dq, :], ps[:], AF.Copy)

                # ---- phase C: attention (K/V streamed from gathered DRAM) ----
                AT = p_at.tile([128, H, T], bfl, name="AT")
                with ExitStack() as s_c:
                    p_mask = s_c.enter_context(tc.tile_pool(name="pmask", bufs=1))
                    p_kh = s_c.enter_context(tc.tile_pool(name="pkh", bufs=3))
                    p_vh = s_c.enter_context(tc.tile_pool(name="pvh", bufs=3))
                    p_pt = s_c.enter_context(tc.tile_pool(name="pt", bufs=3))
                    p_lb = s_c.enter_context(tc.tile_pool(name="lb", bufs=2))
                    maskS = p_mask.tile([128, KC, T], bfl, name="maskS")
                    nc.sync.dma_start(maskS[:], maskT_v[:, :, :])
                    for h in range(H):
                        kh = p_kh.tile([128, TK], bfl, name="kh")
                        nc.sync.dma_start(kh[:], k_out[h // 4, h % 4, :, :])
                        vh = p_vh.tile([128, KC, 128], bfl, name="vh")
                        nc.sync.dma_start(
                            vh[:],
                            v_out[h // 4].rearrange("kc p c -> p kc c")[:, :, (h % 4) * 128:(h % 4 + 1) * 128])
                        av_ps = ps_acc.tile([128, 512], f32, name="av")
                        l_ps = ps_lrow.tile([1, 512], f32, name="lrow")
                        for kc in range(KC):
                            s_ps = ps_mm.tile([128, 512], f32, name="mmps")
                            nc.tensor.matmul(s_ps[:], kh[:, kc * 128:(kc + 1) * 128],
                                             QT[:, h, :], start=True, stop=True)
                            pt = p_pt.tile([128, T], bfl, name="pt")
                            nc.scalar.activation(pt[:], s_ps[:], AF.Exp, scale=ATTN_SCALE)
                            ptm = p_pt.tile([128, T], bfl, name="ptm")
                            nc.vector.tensor_mul(ptm[:], pt[:], maskS[:, kc, :])
                            nc.tensor.matmul(l_ps[:], ones_col_bf[:], ptm[:],
                                             start=(kc == 0), stop=(kc == KC - 1))
                            nc.tensor.matmul(av_ps[:], vh[:, kc, :], ptm[:],
                                             start=(kc == 0), stop=(kc == KC - 1))
                        lrow = p_rows.tile([1, T], f32, name="m_row")
                        nc.vector.tensor_copy(lrow[:], l_ps[:])
                        bc_ps = ps_mm.tile([128, 512], f32, name="mmps")
                        nc.tensor.matmul(bc_ps[:], ones_row_f[:], lrow[:],
                                         start=True, stop=True)
                        lb = p_lb.tile([128, T], f32, name="lbt")
                        nc.vector.reciprocal_approx_fast(lb[:], bc_ps[:])
                        nc.vector.tensor_mul(AT[:, h, :], av_ps[:], lb[:])

            # ---- phase D: o_proj + residual + LN2 ----
            with ExitStack() as s_e:
                p_e = s_e.enter_context(tc.tile_pool(name="pe", bufs=1))
                p_e16 = s_e.enter_context(tc.tile_pool(name="pe16", bufs=16))
                x2T = p_e.tile([128, DB, T], f32, name="x2T")
                x2l = [p_e16.tile([128, T], bfl, name="x2l") for _ in range(DB)]
                with ExitStack() as s_d:
                    p_xo = s_d.enter_context(tc.tile_pool(name="pxo", bufs=4))
                    for do in range(DB):
                        woc = p_wcol.tile([128, DB, 128], bfl, name="wcol")
                        nc.sync.dma_start(woc[:], wo_v[:, :, do * 128:(do + 1) * 128])
                        ps = ps_mm.tile([128, 512], f32, name="mmps")
                        for da in range(DB):
                            nc.tensor.matmul(ps[:], woc[:, da, :], AT[:, da, :],
                                             start=(da == 0), stop=(da == DB - 1))
                        xo = p_xo.tile([128, T], f32, name="xo32")
                        nc.sync.dma_start(xo[:], xqT_v[:, do, :])
                        nc.vector.tensor_add(x2T[:, do, :], ps[:], xo[:])

                    layer_norm(lambda db: x2T[:, db, :], g2t, be2t,
                               lambda db: x2l[db][:], dram_src=False)

                # ---- phase E: MLP ----
                with ExitStack() as s_mlp:
                    p_h1 = s_mlp.enter_context(tc.tile_pool(name="ph1", bufs=1))
                    p_yst = s_mlp.enter_context(tc.tile_pool(name="yst", bufs=3))
                    h1T = p_h1.tile([128, FB, T], bfl, name="h1T")
                    for f in range(FB):
                        w1c = p_wcol.tile([128, DB, 128], bfl, name="wcol")
                        nc.sync.dma_start(w1c[:], w1_v[:, :, f * 128:(f + 1) * 128])
                        ps = ps_mm.tile([128, 512], f32, name="mmps")
                        for db in range(DB):
                            nc.tensor.matmul(ps[:], w1c[:, db, :], x2l[db][:],
                                             start=(db == 0), stop=(db == DB - 1))
                        nc.scalar.activation(h1T[:, f, :], ps[:], AF.Gelu,
                                             bias=b1t[:, f:f + 1])

                    for do in range(DB):
                        ps = ps_acc.tile([128, 512], f32, name="av")
                        for grp in range(4):
                            w2c = p_wcol.tile([128, DB, 128], bfl, name="wcol")
                            nc.sync.dma_start(
                                w2c[:], w2_v[:, grp * DB:(grp + 1) * DB,
                                             do * 128:(do + 1) * 128])
                            for fi in range(DB):
                                fc = grp * DB + fi
                                nc.tensor.matmul(ps[:], w2c[:, fi, :], h1T[:, fc, :],
                                                 start=(fc == 0), stop=(fc == FB - 1))
                        t = p_yst.tile([128, T], f32, name="ycp")
                        nc.scalar.activation(t[:], ps[:], AF.Identity,
                                             bias=b2t[:, do:do + 1])
                        yt = p_yst.tile([128, T], f32, name="yout")
                        nc.vector.tensor_add(yt[:], t[:], x2T[:, do, :])
                        nc.sync.dma_start(yT_v[:, do, :], yt[:])

    nc.compile()
    return nc


def _get_nc():
    if "nc" not in _CACHE:
        _CACHE["nc"] = _build()
    return _CACHE["nc"]


def kernel(x, w_qkv, w_o, w1, b1, w2, b2, g1, be1, g2, be2):
    from concourse.bass_utils import run_bass_kernel_spmd

    nc = _get_nc()

    x = np.asarray(x, np.float32)
    w_qkv = np.asarray(w_qkv)
    wq_b = w_qkv[:, :D].astype(bf16)
    wk_shards = [np.ascontiguousarray(w_qkv[:, D + g * 512: D + (g + 1) * 512]).astype(bf16)
                 for g in range(4)]
    wv_shards = [np.ascontiguousarray(w_qkv[:, 2 * D + g * 512: 2 * D + (g + 1) * 512]).astype(bf16)
                 for g in range(4)]
    wo_b = np.asarray(w_o).astype(bf16)
    w1_b = np.asarray(w1).astype(bf16)
    w2_b = np.asarray(w2).astype(bf16)
    b1T = np.ascontiguousarray(np.asarray(b1, np.float32).reshape(FB, 128).T)
    b2T = np.ascontiguousarray(np.asarray(b2, np.float32).reshape(DB, 128).T)
    g1T = np.ascontiguousarray(np.asarray(g1, np.float32).reshape(DB, 128).T)
    be1T = np.ascontiguousarray(np.asarray(be1, np.float32).reshape(DB, 128).T)
    g2T = np.ascontiguousarray(np.asarray(g2, np.float32).reshape(DB, 128).T)
    be2T = np.ascontiguousarray(np.asarray(be2, np.float32).reshape(DB, 128).T)

    # masks: key j allowed iff j <= (s0 + i) % CHUNK; s0 in {0, 512} mod 1024
    i = np.arange(T)
    j = np.arange(TK)
    masks = {}
    for s0m in (0, 512):
        m = (j[:, None] <= (s0m + i)[None, :]).astype(np.float32)
        masks[s0m] = m.astype(bf16)

    xkvT_b = [np.ascontiguousarray(x[b, :TK].T) for b in range(B)]

    in_maps = []
    for c in range(NC):
        b = c // 4
        s0 = (c % 4) * T
        in_maps.append({
            "xqT": np.ascontiguousarray(x[b, s0:s0 + T].T),
            "xkvT": xkvT_b[b],
            "wq": wq_b, "wk_sh": wk_shards[c % 4], "wv_sh": wv_shards[c % 4],
            "wo": wo_b, "w1": w1_b, "w2": w2_b,
            "maskT": masks[s0 % CHUNK],
            "b1T": b1T, "b2T": b2T, "g1T": g1T, "be1T": be1T,
            "g2T": g2T, "be2T": be2T,
        })

    res = run_bass_kernel_spmd(nc, in_maps, list(range(NC)))

    out = np.empty((B, S, D), np.float32)
    for c in range(NC):
        b = c // 4
        s0 = (c % 4) * T
        out[b, s0:s0 + T] = res.results[c]["yT"].T
    return out
